# revision 4
# baseline (speedup 1.0000x reference)
"""GATv2 x5 (gnn_message_passing) on 8 Trainium2 NeuronCores.

Sharding: nodes partitioned across 8 cores by destination-node owner
(6250 nodes/core, padded to 6272 = 49 tiles of 128). Edges live with
their dst owner, sorted by dst, packed into 128-edge chunks per
dst-tile. Per layer: each core computes xl/xr for its own nodes,
AllGathers xl (the only cross-core exchange), then does
gather -> GATv2 score -> segment softmax -> scatter-add locally via
PE matmuls with runtime-built one-hot matrices.
"""
import sys
import numpy as np

sys.path.insert(0, "/opt/trn_rl_repo")

import concourse.bass as bass
import concourse.bacc as bacc
import concourse.mybir as mybir
import concourse.tile as tile
from concourse.bass_utils import run_bass_kernel_spmd
from concourse.masks import make_identity

F32 = mybir.dt.float32
I32 = mybir.dt.int32
AF = mybir.ActivationFunctionType
OP = mybir.AluOpType

N = 50000
DIN = 7
D = 128
T = 5
CORES = 8
SH = N // CORES            # 6250 nodes per core
TILES = 49
SHP = TILES * 128          # 6272 padded nodes per core
NPG = CORES * SHP          # 50176 global padded node space
NEG = 0.2


def _build_nc(K: int):
    nc = bacc.Bacc("TRN2", target_bir_lowering=False, debug=False,
                   num_devices=CORES)

    xT_full = nc.dram_tensor("xT_full", [DIN, NPG], F32, kind="ExternalInput")
    xT_own = nc.dram_tensor("xT_own", [DIN, SHP], F32, kind="ExternalInput")
    Wlr0 = nc.dram_tensor("Wlr0", [DIN, 2 * D], F32, kind="ExternalInput")
    Wlr = nc.dram_tensor("Wlr", [T - 1, D, 2 * D], F32, kind="ExternalInput")
    brow2 = nc.dram_tensor("brow2", [T, D], F32, kind="ExternalInput")
    bout = nc.dram_tensor("bout", [D, T], F32, kind="ExternalInput")
    attw = nc.dram_tensor("attw", [T, D], F32, kind="ExternalInput")
    src_i = nc.dram_tensor("src_i", [128, TILES * K], I32, kind="ExternalInput")
    dstr_i = nc.dram_tensor("dstr_i", [128, TILES * K], I32, kind="ExternalInput")
    dstl_f = nc.dram_tensor("dstl_f", [128, TILES * K], F32, kind="ExternalInput")

    out_t = nc.dram_tensor("out", [SHP, D], F32, kind="ExternalOutput")

    with tile.TileContext(nc) as tc:
        with (
            tc.tile_pool(name="pers", bufs=1) as pers,
            tc.tile_pool(name="wl", bufs=2) as wl,
            tc.tile_pool(name="edge", bufs=2) as ep,
            tc.tile_pool(name="oh", bufs=4) as ohp,
            tc.tile_pool(name="sb", bufs=3) as sbp,
            tc.tile_pool(name="ps", bufs=2, space="PSUM") as psp,
            tc.tile_pool(name="pst", bufs=2, space="PSUM") as pst,
            tc.tile_pool(name="dram", bufs=2, space="DRAM") as dp,
        ):
            # --- persistent setup ---
            iota_r = pers.tile([128, 128], I32)
            nc.gpsimd.iota(iota_r[:], pattern=[[1, 128]], base=0,
                           channel_multiplier=0)
            ident = pers.tile([128, 128], F32)
            make_identity(nc, ident[:])
            src_sb = pers.tile([128, TILES * K], I32)
            nc.sync.dma_start(out=src_sb[:], in_=src_i[:, :])
            dstr_sb = pers.tile([128, TILES * K], I32)
            nc.sync.dma_start(out=dstr_sb[:], in_=dstr_i[:, :])
            dstl_sb = pers.tile([128, TILES * K], F32)
            nc.sync.dma_start(out=dstl_sb[:], in_=dstl_f[:, :])
            xT_sb = pers.tile([DIN, SHP], F32)
            nc.sync.dma_start(out=xT_sb[:], in_=xT_own[:, :])
            hT = pers.tile([128, SHP], F32)

            for l in range(T):
                xr_dram = dp.tile([SHP, D], F32, tag="xr_dram")
                # --- per-layer constants ---
                w_sb = wl.tile([128, 2 * D], F32, tag="w")
                if l == 0:
                    nc.sync.dma_start(out=w_sb[:DIN, :], in_=Wlr0[:, :])
                else:
                    nc.sync.dma_start(out=w_sb[:], in_=Wlr[l - 1, :, :])
                a_b = wl.tile([128, 128], F32, tag="ab")
                nc.sync.dma_start(
                    out=a_b[:], in_=attw[l : l + 1, :].partition_broadcast(128))
                br2_b = wl.tile([128, 128], F32, tag="br2")
                nc.sync.dma_start(
                    out=br2_b[:], in_=brow2[l : l + 1, :].partition_broadcast(128))
                bo_col = wl.tile([128, 1], F32, tag="boc")
                nc.sync.dma_start(out=bo_col[:], in_=bout[:, l : l + 1])
                if l == T - 1:
                    bo_b = wl.tile([128, 128], F32, tag="bob")
                    nc.sync.dma_start(
                        out=bo_b[:],
                        in_=bout[:, l : l + 1].transpose([1, 0]).partition_broadcast(128))

                # --- prologue: xl_own / xr_own, then AllGather xl ---
                xl_cc = dp.tile([SHP, D], F32, tag="xlcc")
                for m in range(TILES):
                    ps2 = psp.tile([128, 2 * D], F32, space="PSUM", tag="pro")
                    if l == 0:
                        lhsT = xT_sb[:, m * 128 : (m + 1) * 128]
                        rhs = w_sb[:DIN, :]
                    else:
                        lhsT = hT[:, m * 128 : (m + 1) * 128]
                        rhs = w_sb[:, :]
                    nc.tensor.matmul(out=ps2[:], lhsT=lhsT, rhs=rhs,
                                     start=True, stop=True)
                    xl_sb = sbp.tile([128, D], F32, tag="xls")
                    nc.vector.tensor_copy(xl_sb[:], ps2[:, :D])
                    nc.sync.dma_start(
                        out=xl_cc[m * 128 : (m + 1) * 128, :], in_=xl_sb[:])
                    xr_sb = sbp.tile([128, D], F32, tag="xrs")
                    nc.vector.tensor_tensor(
                        out=xr_sb[:], in0=ps2[:, D:], in1=br2_b[:], op=OP.add)
                    nc.sync.dma_start(
                        out=xr_dram[m * 128 : (m + 1) * 128, :], in_=xr_sb[:])

                xl_full = dp.tile([NPG, D], F32, tag="xlfull")
                nc.gpsimd.collective_compute(
                    "AllGather",
                    OP.bypass,
                    replica_groups=[list(range(CORES))],
                    ins=[xl_cc[:, :].opt()],
                    outs=[xl_full[:, :].opt()],
                )

                # --- edge stage: per dst-tile ---
                for t in range(TILES):
                    XL = ep.tile([128, K, D + 1], F32, tag="XL")
                    nc.gpsimd.memset(XL[:, :, D : D + 1], 1.0)
                    XR = ep.tile([128, K, D], F32, tag="XR")
                    for k in range(K):
                        col = t * K + k
                        nc.gpsimd.indirect_dma_start(
                            out=XL[:, k, :D], out_offset=None,
                            in_=xl_full[:, :],
                            in_offset=bass.IndirectOffsetOnAxis(
                                ap=src_sb[:, col : col + 1], axis=0),
                        )
                        nc.gpsimd.indirect_dma_start(
                            out=XR[:, k, :], out_offset=None,
                            in_=xr_dram[:, :],
                            in_offset=bass.IndirectOffsetOnAxis(
                                ap=dstr_sb[:, col : col + 1], axis=0),
                        )
                    S = ep.tile([128, K, D], F32, tag="S")
                    nc.vector.tensor_tensor(
                        out=S[:, :, :], in0=XL[:, :, :D], in1=XR[:, :, :],
                        op=OP.add)
                    L = ep.tile([128, K, D], F32, tag="L")
                    nc.scalar.activation(
                        out=L[:, :, :], in_=S[:, :, :], func=AF.Prelu, alpha=NEG)
                    e_t = sbp.tile([128, K], F32, tag="e")
                    scr = sbp.tile([128, 128], F32, tag="scr")
                    for k in range(K):
                        nc.vector.scalar_tensor_tensor(
                            out=scr[:], in0=L[:, k, :], scalar=1.0,
                            in1=a_b[:], op0=OP.mult, op1=OP.mult,
                            accum_out=e_t[:, k : k + 1])
                    ex_t = sbp.tile([128, K], F32, tag="ex")
                    nc.scalar.activation(out=ex_t[:], in_=e_t[:], func=AF.Exp)

                    ps_a = pst.tile([128, D + 1], F32, space="PSUM", tag="agg")
                    for k in range(K):
                        col = t * K + k
                        Oc = ohp.tile([128, 128], F32, tag="O")
                        nc.vector.tensor_scalar(
                            out=Oc[:], in0=iota_r[:],
                            scalar1=dstl_sb[:, col : col + 1],
                            scalar2=ex_t[:, k : k + 1],
                            op0=OP.is_equal, op1=OP.mult)
                        nc.tensor.matmul(
                            out=ps_a[:], lhsT=Oc[:], rhs=XL[:, k, :],
                            start=(k == 0), stop=(k == K - 1))

                    rec = sbp.tile([128, 1], F32, tag="rec")
                    nc.vector.reciprocal(rec[:], ps_a[:, D : D + 1])
                    h_sb = sbp.tile([128, D], F32, tag="h")
                    nc.vector.tensor_scalar(
                        out=h_sb[:], in0=ps_a[:, :D], scalar1=rec[:],
                        scalar2=None, op0=OP.mult)
                    if l < T - 1:
                        ps_t = pst.tile([128, 128], F32, space="PSUM", tag="tr")
                        nc.tensor.transpose(out=ps_t[:], in_=h_sb[:],
                                            identity=ident[:])
                        nc.scalar.activation(
                            out=hT[:, t * 128 : (t + 1) * 128], in_=ps_t[:],
                            func=AF.Relu, bias=bo_col[:], scale=1.0)
                    else:
                        o_sb = sbp.tile([128, D], F32, tag="o")
                        nc.vector.tensor_tensor(
                            out=o_sb[:], in0=h_sb[:], in1=bo_b[:], op=OP.add)
                        nc.sync.dma_start(
                            out=out_t[t * 128 : (t + 1) * 128, :], in_=o_sb[:])

    nc.compile()
    return nc


def _prep(inputs):
    x = np.asarray(inputs["x"], np.float32)
    ei = np.asarray(inputs["edge_index"]).astype(np.int64)
    Wl0 = np.asarray(inputs["Wl0"], np.float32)
    Wr0 = np.asarray(inputs["Wr0"], np.float32)
    bl0 = np.asarray(inputs["bl0"], np.float32)
    br0 = np.asarray(inputs["br0"], np.float32)
    Wl = np.asarray(inputs["Wl"], np.float32)
    Wr = np.asarray(inputs["Wr"], np.float32)
    bl = np.asarray(inputs["bl"], np.float32)
    br = np.asarray(inputs["br"], np.float32)
    att = np.asarray(inputs["att"], np.float32)
    bias = np.asarray(inputs["bias"], np.float32)

    loop = np.arange(N, dtype=np.int64)
    src = np.concatenate([ei[0], loop])
    dst = np.concatenate([ei[1], loop])

    owner = dst // SH
    local = dst - owner * SH

    # global padded row of each src node
    gsrc = (src // SH) * SHP + (src % SH)

    per_core = []
    max_cnt = 0
    for c in range(CORES):
        sel = owner == c
        s_g = gsrc[sel]
        s_loc = local[sel]
        order = np.argsort(s_loc, kind="stable")
        s_g = s_g[order]
        s_loc = s_loc[order]
        tid = s_loc >> 7
        counts = np.bincount(tid, minlength=TILES).astype(np.int64)
        # fake self-edges for pad nodes (local 6250..6271 -> tile 48)
        counts[TILES - 1] += SHP - SH
        max_cnt = max(max_cnt, int(counts.max()))
        per_core.append((s_g, s_loc, tid, counts))

    K = int(np.ceil(max_cnt / 128))

    srcs, dstrs, dstls = [], [], []
    for c in range(CORES):
        s_g, s_loc, tid, counts = per_core[c]
        src_arr = np.zeros((128, TILES * K), np.int32)
        dstr_arr = np.zeros((128, TILES * K), np.int32)
        dstl_arr = np.full((128, TILES * K), 200.0, np.float32)
        bounds = np.concatenate([[0], np.cumsum(np.bincount(tid, minlength=TILES))])
        for t in range(TILES):
            seg = slice(bounds[t], bounds[t + 1])
            n_e = bounds[t + 1] - bounds[t]
            e_g = s_g[seg]
            e_loc = s_loc[seg] & 127
            e_dstrow = s_loc[seg]
            if t == TILES - 1:
                # pad-node fake self-edges keep denominators nonzero
                pads = np.arange(SH, SHP, dtype=np.int64)
                e_g = np.concatenate([e_g, np.zeros(SHP - SH, np.int64)])
                e_loc = np.concatenate([e_loc, pads & 127])
                e_dstrow = np.concatenate([e_dstrow, pads])
                n_e += SHP - SH
            slot = np.arange(n_e)
            p = slot & 127
            k = slot >> 7
            src_arr[p, t * K + k] = e_g
            dstr_arr[p, t * K + k] = e_dstrow
            dstl_arr[p, t * K + k] = e_loc
        srcs.append(src_arr)
        dstrs.append(dstr_arr)
        dstls.append(dstl_arr)

    # weight / bias packing (biases folded: xl is bias-free, xr carries
    # bl+br for the score, output carries bias+bl)
    Wlr0 = np.concatenate([Wl0, Wr0], axis=1)
    Wlr = np.concatenate([Wl, Wr], axis=2)
    brow2 = np.stack([bl0 + br0] + [bl[i] + br[i] for i in range(T - 1)])
    bout = np.stack([bias[0] + bl0] + [bias[i + 1] + bl[i] for i in range(T - 1)]).T.copy()

    xT_full = np.zeros((DIN, NPG), np.float32)
    for c in range(CORES):
        xT_full[:, c * SHP : c * SHP + SH] = x[c * SH : (c + 1) * SH].T

    common = dict(Wlr0=Wlr0, Wlr=Wlr, brow2=brow2, bout=bout, attw=att,
                  xT_full=xT_full)
    in_maps = []
    for c in range(CORES):
        xT_own = np.zeros((DIN, SHP), np.float32)
        xT_own[:, :SH] = x[c * SH : (c + 1) * SH].T
        in_maps.append(dict(common, xT_own=xT_own, src_i=srcs[c],
                            dstr_i=dstrs[c], dstl_f=dstls[c]))
    return K, in_maps


_CACHE = {}


def kernel(**inputs) -> np.ndarray:
    out, _ = _run(inputs)
    return out


def _run(inputs, **kw):
    K, in_maps = _prep(inputs)
    if K not in _CACHE:
        _CACHE[K] = _build_nc(K)
    nc = _CACHE[K]
    res = run_bass_kernel_spmd(nc, in_maps, core_ids=list(range(CORES)), **kw)
    out = np.concatenate([res.results[c]["out"][:SH] for c in range(CORES)], axis=0)
    return out.astype(np.float32), res
